# revision 15
# baseline (speedup 1.0000x reference)
"""Multi-head self-attention Trainium2 Bass kernel.

Problem: B=4, T=2048, EMB=1024, H=16 heads (head_dim 64), causal mask,
scores scaled by 1/sqrt(EMB), torch-Linear style projections.

Sharding (8 cores): data-parallel over the 4 batches x tensor-parallel over
2 head-groups of 8 heads.  Core c handles batch c//2, heads (c%2)*8..+8.
Each core computes q/k/v projections for its head shard, full TxT causal
attention for its 8 heads, and a partial output projection (its 512 rows of
the unify matmul).  Host sums the two partial outputs per batch and adds bo.

Device layout notes:
 - All PE operands are fp16 (1 cycle/row on the PE); PSUM accumulates fp32.
 - x and the weights are pre-transposed/cast on host so that every matmul
   contraction dim lands on the SBUF partition axis.
 - Scores are computed transposed (pT[s, t] = exp(q.k/32)) so that the
   attn @ v contraction (over s) needs no on-device transposes.  Softmax
   sums ride as a 65th "ones" column of v; normalization happens on the
   small yT tensor via reciprocal + gpsimd partition-broadcast.
"""

import numpy as np

B, T, EMB, H = 4, 2048, 1024, 16
HD = 64          # head dim
HPC = 8          # heads per core
DPC = HPC * HD   # projected dim per core = 512
NCORES = 8
E8 = EMB // 128  # contraction chunks over emb = 8
DP = DPC // 128  # head-pair chunks = 4
TB = T // 512    # t-blocks = 4
SC = T // 128    # s-chunks = 16
VW = HD + 1      # v columns per head incl. ones column = 65
GROUP = 3        # score chunks per exp group (3 PSUM banks)

_CACHED_NC = None
LAST_RESULTS = None  # BassKernelResults of the most recent run (for test.py)


def _build_nc():
    import concourse.bacc as bacc
    import concourse.tile as tile
    import concourse.mybir as mybir

    f16 = mybir.dt.float16
    f32 = mybir.dt.float32
    Exp = mybir.ActivationFunctionType.Exp

    nc = bacc.Bacc(
        "TRN2",
        target_bir_lowering=False,
        debug=False,
        enable_asserts=False,
        num_devices=NCORES,
    )

    xT_d = nc.dram_tensor("xT", [EMB, T], f16, kind="ExternalInput").ap()
    wqT_d = nc.dram_tensor("wqT", [EMB, DPC], f16, kind="ExternalInput").ap()
    wkT_d = nc.dram_tensor("wkT", [EMB, DPC], f16, kind="ExternalInput").ap()
    wvT_d = nc.dram_tensor("wvT", [EMB, DPC], f16, kind="ExternalInput").ap()
    woT_d = nc.dram_tensor("woT", [DPC, EMB], f16, kind="ExternalInput").ap()
    cm_d = nc.dram_tensor("cmask", [128, 2560], f16, kind="ExternalInput").ap()
    out_d = nc.dram_tensor("out", [T, EMB], f32, kind="ExternalOutput").ap()

    with tile.TileContext(nc) as tc:
        # ---- persistent SBUF tensors (static allocations) -------------
        def sb(name, shape):
            return nc.alloc_sbuf_tensor(name, list(shape), f16).ap()

        xt = [sb(f"xt{k}", [128, T]) for k in range(E8)]
        wq = [sb(f"wq{k}", [128, DPC]) for k in range(E8)]
        wk = [sb(f"wk{k}", [128, DPC]) for k in range(E8)]
        wv = [sb(f"wv{k}", [128, DPC]) for k in range(E8)]
        wo = [sb(f"wo{p}", [128, EMB]) for p in range(DP)]
        cm = sb("cm", [128, 2560])
        qt = [sb(f"qt{p}", [128, T]) for p in range(DP)]
        kt = [sb(f"kt{p}", [128, T]) for p in range(DP)]
        vt = sb("vt", [128, SC * HPC * VW])
        ytn = [sb(f"ytn{p}", [128, T]) for p in range(DP)]

        # ---- input DMAs ------------------------------------------------
        # Loads are staged DRAM -> staging tile -> DVE copy -> final tile so
        # every matmul operand's producer is on the DVE clock (the MM ISA
        # struct has room for very few sync-wait commands; keeping all
        # operand deps on one engine's semaphore collapses them to one).
        with tc.tile_pool(name="stage", bufs=4) as stg:
            def load(dst, src):
                s = stg.tile(list(dst.shape), f16, tag="stage",
                             name=f"st_{dst.tensor.name}")
                nc.sync.dma_start(s[:, :], src)
                nc.vector.tensor_copy(dst, s[:, :])

            load(cm[:, :], cm_d[:, :])
            for k in range(E8):
                r = slice(k * 128, (k + 1) * 128)
                load(xt[k][:, :], xT_d[r, :])
                load(wq[k][:, :], wqT_d[r, :])
                load(wk[k][:, :], wkT_d[r, :])
                load(wv[k][:, :], wvT_d[r, :])
            for p in range(DP):
                load(wo[p][:, :], woT_d[p * 128:(p + 1) * 128, :])
        # ones columns for the softmax-sum trick (data cols overwritten below)
        nc.vector.memset(vt[:, :], 1.0)

        # ---- phase 1: q/k/v projections -------------------------------
        with tc.tile_pool(name="pp", bufs=4, space="PSUM") as pp:
            for p in range(DP):
                dcols = slice(p * 128, (p + 1) * 128)
                for j in range(TB):
                    tcols = slice(j * 512, (j + 1) * 512)
                    for w_t, dst in ((wq, qt), (wk, kt)):
                        ps = pp.tile([128, 512], f32, tag="pp", name=f"ps_{p}_{j}")
                        for e in range(E8):
                            nc.tensor.matmul(
                                ps[:, :],
                                w_t[e][:, dcols],
                                xt[e][:, tcols],
                                start=(e == 0),
                                stop=(e == E8 - 1),
                            )
                        nc.vector.tensor_copy(dst[p][:, tcols], ps[:, :])
            for s in range(SC):
                ps = pp.tile([128, 512], f32, tag="pp", name=f"psv_{s}")
                for e in range(E8):
                    nc.tensor.matmul(
                        ps[:, :],
                        xt[e][:, s * 128:(s + 1) * 128],
                        wv[e][:, :],
                        start=(e == 0),
                        stop=(e == E8 - 1),
                    )
                dst = vt[:, s * HPC * VW:(s + 1) * HPC * VW]
                dst = dst.rearrange("p (h c) -> p h c", c=VW)[:, :, 0:HD]
                src = ps[:, :].rearrange("p (h c) -> p h c", c=HD)
                nc.vector.tensor_copy(dst, src)

        # ---- phase 2: attention ---------------------------------------
        with (
            tc.tile_pool(name="scp", bufs=2, space="PSUM") as scp,
            tc.tile_pool(name="ytp", bufs=1, space="PSUM") as ytp,
            tc.tile_pool(name="ptp", bufs=3) as ptp,
            tc.tile_pool(name="recp", bufs=2) as recp,
            tc.tile_pool(name="brecp", bufs=2) as brecp,
        ):
            for p in range(DP):
                for j in range(TB):
                    tcols = slice(j * 512, (j + 1) * 512)
                    nchunks = 4 * j + 4
                    yts = [
                        ytp.tile([VW, 512], f32, tag=f"yt{h2}", name=f"yt{h2}_{p}_{j}")
                        for h2 in range(2)
                    ]
                    groups = [
                        list(range(g, min(g + GROUP, nchunks)))
                        for g in range(0, nchunks, GROUP)
                    ]
                    for cks in groups:
                        w = 512 * len(cks)
                        for h2 in range(2):
                            base = h2 * 64
                            h = 2 * p + h2
                            sc = scp.tile([128, w], f32, tag="sc",
                                          name=f"sc{h2}_{p}_{j}_{cks[0]}")
                            for i, ck in enumerate(cks):
                                nc.tensor.matmul(
                                    sc[:, i * 512:(i + 1) * 512],
                                    kt[p][base:base + 64, ck * 128:(ck + 1) * 128],
                                    qt[p][base:base + 64, tcols],
                                    start=True,
                                    stop=True,
                                )
                            pt = ptp.tile([128, w], f16, tag="pt",
                                          name=f"pt{h2}_{p}_{j}_{cks[0]}")
                            nc.scalar.activation(pt[:, :], sc[:, :], Exp,
                                                 scale=1.0 / 32.0)
                            for i, ck in enumerate(cks):
                                di = ck - 4 * j
                                # Diagonal chunks need the causal mask; the
                                # first accumulated chunk gets an all-ones
                                # "mask" (block 4) so the accumulation-start
                                # matmul's deps land on the DVE clock.
                                if di < 0 and ck == 0:
                                    di = 4
                                if di >= 0:
                                    pcols = slice(i * 512, (i + 1) * 512)
                                    nc.vector.tensor_mul(
                                        pt[:, pcols],
                                        pt[:, pcols],
                                        cm[:, di * 512:(di + 1) * 512],
                                    )
                            for i, ck in enumerate(cks):
                                nc.tensor.matmul(
                                    yts[h2][:, :],
                                    vt[:, ck * HPC * VW + h * VW:
                                       ck * HPC * VW + (h + 1) * VW],
                                    pt[:, i * 512:(i + 1) * 512],
                                    start=(ck == 0),
                                    stop=(ck == nchunks - 1),
                                )
                    for h2 in range(2):
                        base = h2 * 64
                        rec = recp.tile([1, 512], f32, tag="rec",
                                        name=f"rec{h2}_{p}_{j}")
                        nc.vector.reciprocal(rec[:, :], yts[h2][HD:HD + 1, :])
                        brec = brecp.tile([64, 512], f32, tag="brec",
                                          name=f"brec{h2}_{p}_{j}")
                        nc.gpsimd.partition_broadcast(brec[:, :], rec[:, :])
                        nc.vector.tensor_mul(
                            ytn[p][base:base + 64, tcols],
                            yts[h2][0:HD, :],
                            brec[:, :],
                        )

        # ---- phase 3: output projection (partial over e_in) -----------
        with (
            tc.tile_pool(name="opp", bufs=4, space="PSUM") as opp,
            tc.tile_pool(name="ost", bufs=4) as ost,
        ):
            for tcn in range(T // 128):
                trows = slice(tcn * 128, (tcn + 1) * 128)
                for n in range(EMB // 512):
                    ncols = slice(n * 512, (n + 1) * 512)
                    ps = opp.tile([128, 512], f32, tag="op", name=f"op_{tcn}_{n}")
                    for p in range(DP):
                        nc.tensor.matmul(
                            ps[:, :],
                            ytn[p][:, trows],
                            wo[p][:, ncols],
                            start=(p == 0),
                            stop=(p == DP - 1),
                        )
                    ot = ost.tile([128, 512], f32, tag="ot", name=f"ot_{tcn}_{n}")
                    nc.vector.tensor_copy(ot[:, :], ps[:, :])
                    nc.sync.dma_start(out_d[trows, ncols], ot[:, :])

    nc.compile()
    return nc


def _causal_mask_tiles() -> np.ndarray:
    """[128, 2560] fp16: tile i<4 (cols 512i..) is the mask for diagonal
    s-chunk offset i: m[p, c] = 1 if 128*i + p <= c else 0.  Tile 4 is all
    ones (used as a dep-shaping no-op multiply)."""
    m = np.zeros((128, 5, 512), dtype=np.float16)
    p = np.arange(128)[:, None]
    c = np.arange(512)[None, :]
    for i in range(4):
        m[:, i, :] = (128 * i + p <= c).astype(np.float16)
    m[:, 4, :] = 1.0
    return np.ascontiguousarray(m.reshape(128, 2560))


def _numpy_fallback(x, mask, Wq, bq, Wk, bk, Wv, bv, Wo, bo):
    b, t, emb = x.shape
    h = H
    k = emb // h
    q = (x @ Wq.T + bq).reshape(b, t, h, k)
    kk = (x @ Wk.T + bk).reshape(b, t, h, k)
    v = (x @ Wv.T + bv).reshape(b, t, h, k)
    scale = 1.0 / np.sqrt(emb)
    out = np.empty((b, t, emb), dtype=np.float32)
    for bi in range(b):
        yb = np.empty((t, h, k), dtype=np.float32)
        for hi in range(h):
            s = (q[bi, :, hi] @ kk[bi, :, hi].T) * scale
            s = np.where(mask[bi] == 0, np.float32(-1e10), s)
            s = s - s.max(axis=-1, keepdims=True)
            e = np.exp(s)
            p = e / e.sum(axis=-1, keepdims=True)
            yb[:, hi] = p @ v[bi, :, hi]
        out[bi] = yb.reshape(t, emb) @ Wo.T + bo
    return out


def kernel(x, mask, Wq, bq, Wk, bk, Wv, bv, Wo, bo):
    global _CACHED_NC, LAST_RESULTS
    x = np.asarray(x, dtype=np.float32)
    mask = np.asarray(mask)
    Wq, Wk, Wv, Wo = (np.asarray(w, dtype=np.float32) for w in (Wq, Wk, Wv, Wo))
    bq, bk, bv, bo = (np.asarray(v_, dtype=np.float32) for v_ in (bq, bk, bv, bo))

    # The device program hardcodes a causal mask and zero q/k/v biases
    # (which is what reference.setup_inputs produces).  Anything else falls
    # back to a plain numpy implementation.
    tril = np.tril(np.ones((T, T), dtype=mask.dtype))
    if (
        x.shape != (B, T, EMB)
        or any(np.any(bias) for bias in (bq, bk, bv))
        or not all(np.array_equal(np.asarray(mask[b_]), tril) for b_ in range(B))
    ):
        return _numpy_fallback(x, mask, Wq, bq, Wk, bk, Wv, bv, Wo, bo)

    from concourse import bass_utils

    f16 = np.float16
    xT = [np.ascontiguousarray(x[b_].T).astype(f16) for b_ in range(B)]
    cmask = _causal_mask_tiles()
    in_maps = []
    for c in range(NCORES):
        b_, hg = c // 2, c % 2
        r = slice(hg * DPC, (hg + 1) * DPC)
        in_maps.append({
            "xT": xT[b_],
            "wqT": np.ascontiguousarray(Wq[r, :].T).astype(f16),
            "wkT": np.ascontiguousarray(Wk[r, :].T).astype(f16),
            "wvT": np.ascontiguousarray(Wv[r, :].T).astype(f16),
            "woT": np.ascontiguousarray(Wo[:, r].T).astype(f16),
            "cmask": cmask,
        })

    if _CACHED_NC is None:
        _CACHED_NC = _build_nc()

    import os
    trace = bool(int(os.environ.get("KERNEL_TRACE", "0")))
    res = bass_utils.run_bass_kernel_spmd(
        _CACHED_NC,
        in_maps,
        core_ids=list(range(NCORES)),
        trace=trace,
    )
    LAST_RESULTS = res
    outs = [r["out"] for r in res.results]
    y = np.stack([outs[2 * b_] + outs[2 * b_ + 1] for b_ in range(B)])
    y += bo[None, None, :]
    return np.ascontiguousarray(y.astype(np.float32))


# revision 19
# speedup vs baseline: 1.1285x; 1.1285x over previous
"""Multi-head self-attention Trainium2 Bass kernel.

Problem: B=4, T=2048, EMB=1024, H=16 heads (head_dim 64), causal mask,
scores scaled by 1/sqrt(EMB), torch-Linear style projections.

Sharding (8 cores): data-parallel over the 4 batches x tensor-parallel over
2 head-groups of 8 heads.  Core c handles batch c//2, heads (c%2)*8..+8.
Each core computes q/k/v projections for its head shard, full TxT causal
attention for its 8 heads, and a partial output projection (its 512 rows of
the unify matmul).  Host sums the two partial outputs per batch and adds bo.

Device layout notes:
 - All PE operands are fp16 (1 cycle/row on the PE); PSUM accumulates fp32.
 - x and the weights are pre-transposed/cast on host so that every matmul
   contraction dim lands on the SBUF partition axis.
 - Scores are computed transposed (pT[s, t] = exp(q.k/32)) so that the
   attn @ v contraction (over s) needs no on-device transposes.  Softmax
   sums ride as a 65th "ones" column of v; normalization happens on the
   small yT tensor via reciprocal + gpsimd partition-broadcast.
"""

import numpy as np

B, T, EMB, H = 4, 2048, 1024, 16
HD = 64          # head dim
HPC = 8          # heads per core
DPC = HPC * HD   # projected dim per core = 512
NCORES = 8
E8 = EMB // 128  # contraction chunks over emb = 8
DP = DPC // 128  # head-pair chunks = 4
TB = T // 512    # t-blocks = 4
SC = T // 128    # s-chunks = 16
VW = HD + 1      # v columns per head incl. ones column = 65
GROUP = 3        # score chunks per exp group (3 PSUM banks)

_CACHED_NC = None
LAST_RESULTS = None  # BassKernelResults of the most recent run (for test.py)


def _build_nc():
    import concourse.bacc as bacc
    import concourse.tile as tile
    import concourse.mybir as mybir

    f16 = mybir.dt.float16
    f32 = mybir.dt.float32
    Exp = mybir.ActivationFunctionType.Exp

    nc = bacc.Bacc(
        "TRN2",
        target_bir_lowering=False,
        debug=False,
        enable_asserts=False,
        num_devices=NCORES,
    )

    xT_d = nc.dram_tensor("xT", [EMB, T], f16, kind="ExternalInput").ap()
    wqT_d = nc.dram_tensor("wqT", [EMB, DPC], f16, kind="ExternalInput").ap()
    wkT_d = nc.dram_tensor("wkT", [EMB, DPC], f16, kind="ExternalInput").ap()
    wvT_d = nc.dram_tensor("wvT", [EMB, DPC], f16, kind="ExternalInput").ap()
    woT_d = nc.dram_tensor("woT", [DPC, EMB], f16, kind="ExternalInput").ap()
    cm_d = nc.dram_tensor("cmask", [128, 2560], f16, kind="ExternalInput").ap()
    out_d = nc.dram_tensor("out", [T, EMB], f32, kind="ExternalOutput").ap()

    with tile.TileContext(nc) as tc:
        # ---- persistent SBUF tensors (static allocations) -------------
        def sb(name, shape):
            return nc.alloc_sbuf_tensor(name, list(shape), f16).ap()

        xt = [sb(f"xt{k}", [128, T]) for k in range(E8)]
        wq = [sb(f"wq{k}", [128, DPC]) for k in range(E8)]
        wk = [sb(f"wk{k}", [128, DPC]) for k in range(E8)]
        wv = [sb(f"wv{k}", [128, DPC]) for k in range(E8)]
        wo = [sb(f"wo{p}", [128, EMB]) for p in range(DP)]
        cm = sb("cm", [128, 2560])
        qt = [sb(f"qt{p}", [128, T]) for p in range(DP)]
        kt = [sb(f"kt{p}", [128, T]) for p in range(DP)]
        vt = sb("vt", [128, SC * HPC * VW])
        ytn = [sb(f"ytn{p}", [128, T]) for p in range(DP)]

        # ---- input DMAs ------------------------------------------------
        # Loads are staged DRAM -> staging tile -> DVE copy -> final tile so
        # every matmul operand's producer is on the DVE clock (the MM ISA
        # struct has room for very few sync-wait commands; keeping all
        # operand deps on one engine's semaphore collapses them to one).
        with tc.tile_pool(name="stage", bufs=4) as stg:
            def load(dst, src):
                s = stg.tile(list(dst.shape), f16, tag="stage",
                             name=f"st_{dst.tensor.name}")
                nc.sync.dma_start(s[:, :], src)
                nc.vector.tensor_copy(dst, s[:, :])

            load(cm[:, :], cm_d[:, :])
            for k in range(E8):
                r = slice(k * 128, (k + 1) * 128)
                load(xt[k][:, :], xT_d[r, :])
                load(wq[k][:, :], wqT_d[r, :])
                load(wk[k][:, :], wkT_d[r, :])
                load(wv[k][:, :], wvT_d[r, :])
            for p in range(DP):
                load(wo[p][:, :], woT_d[p * 128:(p + 1) * 128, :])
        # ones columns for the softmax-sum trick (data cols overwritten below)
        nc.vector.memset(vt[:, :], 1.0)

        # ---- phase 1: q/k/v projections -------------------------------
        with tc.tile_pool(name="pp", bufs=4, space="PSUM") as pp:
            for p in range(DP):
                dcols = slice(p * 128, (p + 1) * 128)
                for j in range(TB):
                    tcols = slice(j * 512, (j + 1) * 512)
                    for w_t, dst in ((wq, qt), (wk, kt)):
                        ps = pp.tile([128, 512], f32, tag="pp", name=f"ps_{p}_{j}")
                        for e in range(E8):
                            nc.tensor.matmul(
                                ps[:, :],
                                w_t[e][:, dcols],
                                xt[e][:, tcols],
                                start=(e == 0),
                                stop=(e == E8 - 1),
                            )
                        nc.vector.tensor_copy(dst[p][:, tcols], ps[:, :])
            for s in range(SC):
                ps = pp.tile([128, 512], f32, tag="pp", name=f"psv_{s}")
                for e in range(E8):
                    nc.tensor.matmul(
                        ps[:, :],
                        xt[e][:, s * 128:(s + 1) * 128],
                        wv[e][:, :],
                        start=(e == 0),
                        stop=(e == E8 - 1),
                    )
                dst = vt[:, s * HPC * VW:(s + 1) * HPC * VW]
                dst = dst.rearrange("p (h c) -> p h c", c=VW)[:, :, 0:HD]
                src = ps[:, :].rearrange("p (h c) -> p h c", c=HD)
                nc.vector.tensor_copy(dst, src)

        # ---- phase 2: attention ---------------------------------------
        # j outer so each t-block's softmax sums complete together; the
        # normalization (reciprocal / broadcast / scale) is deferred off the
        # PE critical path: yT and sums are evacuated from PSUM with two
        # quick DVE copies so the yt PSUM slot recycles immediately.
        with (
            tc.tile_pool(name="scp", bufs=2, space="PSUM") as scp,
            tc.tile_pool(name="ytp", bufs=1, space="PSUM") as ytp,
            tc.tile_pool(name="ptp", bufs=6) as ptp,
            tc.tile_pool(name="sump", bufs=2) as sump,
            tc.tile_pool(name="recp", bufs=2) as recp,
            tc.tile_pool(name="rrow", bufs=2) as rrow,
            tc.tile_pool(name="brecp", bufs=2) as brecp,
        ):
            for j in range(TB):
                tcols = slice(j * 512, (j + 1) * 512)
                nchunks = 4 * j + 4
                # Single-partition writes must start at a 32-aligned
                # partition, so the 8 sums rows live at partitions
                # {0,32,64,96} of two tiles.
                sums = [
                    sump.tile([97, 512], f32, tag=f"sums{t_}", name=f"sums{t_}_{j}")
                    for t_ in range(2)
                ]
                for t_ in range(2):
                    nc.vector.memset(sums[t_][:, :], 1.0)
                groups = [
                    list(range(g, min(g + GROUP, nchunks)))
                    for g in range(0, nchunks, GROUP)
                ]
                for p in range(DP):
                    yts = [
                        ytp.tile([VW, 512], f32, tag=f"yt{h2}", name=f"yt{h2}_{p}_{j}")
                        for h2 in range(2)
                    ]
                    for cks in groups:
                        w = 512 * len(cks)
                        for h2 in range(2):
                            base = h2 * 64
                            h = 2 * p + h2
                            sc = scp.tile([128, w], f32, tag="sc",
                                          name=f"sc{h2}_{p}_{j}_{cks[0]}")
                            for i, ck in enumerate(cks):
                                nc.tensor.matmul(
                                    sc[:, i * 512:(i + 1) * 512],
                                    kt[p][base:base + 64, ck * 128:(ck + 1) * 128],
                                    qt[p][base:base + 64, tcols],
                                    start=True,
                                    stop=True,
                                )
                            pt = ptp.tile([128, w], f16, tag="pt",
                                          name=f"pt{h2}_{p}_{j}_{cks[0]}")
                            nc.scalar.activation(pt[:, :], sc[:, :], Exp,
                                                 scale=1.0 / 32.0)
                            for i, ck in enumerate(cks):
                                di = ck - 4 * j
                                # Diagonal chunks need the causal mask; the
                                # first accumulated chunk gets an all-ones
                                # "mask" (block 4) so the accumulation-start
                                # matmul's deps land on the DVE clock.
                                if di < 0 and ck == 0:
                                    di = 4
                                if di >= 0:
                                    pcols = slice(i * 512, (i + 1) * 512)
                                    nc.vector.tensor_mul(
                                        pt[:, pcols],
                                        pt[:, pcols],
                                        cm[:, di * 512:(di + 1) * 512],
                                    )
                            for i, ck in enumerate(cks):
                                nc.tensor.matmul(
                                    yts[h2][:, :],
                                    vt[:, ck * HPC * VW + h * VW:
                                       ck * HPC * VW + (h + 1) * VW],
                                    pt[:, i * 512:(i + 1) * 512],
                                    start=(ck == 0),
                                    stop=(ck == nchunks - 1),
                                )
                    for h2 in range(2):
                        base = h2 * 64
                        r = p * 2 + h2
                        # quick PSUM evacuation: unnormalized yT + sums row
                        nc.vector.tensor_copy(ytn[p][base:base + 64, tcols],
                                              yts[h2][0:HD, :])
                        row = 32 * (r % 4)
                        nc.vector.tensor_copy(sums[r // 4][row:row + 1, :],
                                              yts[h2][HD:HD + 1, :])
                # deferred normalization for this t-block (overlaps next j)
                rec = [
                    recp.tile([97, 512], f32, tag=f"rec{t_}", name=f"rec{t_}_{j}")
                    for t_ in range(2)
                ]
                for t_ in range(2):
                    nc.vector.reciprocal(rec[t_][:, :], sums[t_][:, :])
                for p in range(DP):
                    for h2 in range(2):
                        base = h2 * 64
                        r = p * 2 + h2
                        row = 32 * (r % 4)
                        rr = rrow.tile([1, 512], f32, tag="rr",
                                       name=f"rr{h2}_{p}_{j}")
                        nc.vector.tensor_copy(rr[:, :], rec[r // 4][row:row + 1, :])
                        # both SBUF operands of tensor_tensor must share a
                        # base partition: broadcast to all 128 and slice.
                        brec = brecp.tile([128, 512], f32, tag="brec",
                                          name=f"brec{h2}_{p}_{j}")
                        nc.gpsimd.partition_broadcast(brec[:, :], rr[:, :])
                        nc.vector.tensor_mul(
                            ytn[p][base:base + 64, tcols],
                            ytn[p][base:base + 64, tcols],
                            brec[base:base + 64, :],
                        )

        # ---- phase 3: output projection (partial over e_in) -----------
        with (
            tc.tile_pool(name="opp", bufs=4, space="PSUM") as opp,
            tc.tile_pool(name="ost", bufs=4) as ost,
        ):
            for tcn in range(T // 128):
                trows = slice(tcn * 128, (tcn + 1) * 128)
                for n in range(EMB // 512):
                    ncols = slice(n * 512, (n + 1) * 512)
                    ps = opp.tile([128, 512], f32, tag="op", name=f"op_{tcn}_{n}")
                    for p in range(DP):
                        nc.tensor.matmul(
                            ps[:, :],
                            ytn[p][:, trows],
                            wo[p][:, ncols],
                            start=(p == 0),
                            stop=(p == DP - 1),
                        )
                    ot = ost.tile([128, 512], f32, tag="ot", name=f"ot_{tcn}_{n}")
                    nc.vector.tensor_copy(ot[:, :], ps[:, :])
                    nc.sync.dma_start(out_d[trows, ncols], ot[:, :])

    nc.compile()
    return nc


def _causal_mask_tiles() -> np.ndarray:
    """[128, 2560] fp16: tile i<4 (cols 512i..) is the mask for diagonal
    s-chunk offset i: m[p, c] = 1 if 128*i + p <= c else 0.  Tile 4 is all
    ones (used as a dep-shaping no-op multiply)."""
    m = np.zeros((128, 5, 512), dtype=np.float16)
    p = np.arange(128)[:, None]
    c = np.arange(512)[None, :]
    for i in range(4):
        m[:, i, :] = (128 * i + p <= c).astype(np.float16)
    m[:, 4, :] = 1.0
    return np.ascontiguousarray(m.reshape(128, 2560))


def _numpy_fallback(x, mask, Wq, bq, Wk, bk, Wv, bv, Wo, bo):
    b, t, emb = x.shape
    h = H
    k = emb // h
    q = (x @ Wq.T + bq).reshape(b, t, h, k)
    kk = (x @ Wk.T + bk).reshape(b, t, h, k)
    v = (x @ Wv.T + bv).reshape(b, t, h, k)
    scale = 1.0 / np.sqrt(emb)
    out = np.empty((b, t, emb), dtype=np.float32)
    for bi in range(b):
        yb = np.empty((t, h, k), dtype=np.float32)
        for hi in range(h):
            s = (q[bi, :, hi] @ kk[bi, :, hi].T) * scale
            s = np.where(mask[bi] == 0, np.float32(-1e10), s)
            s = s - s.max(axis=-1, keepdims=True)
            e = np.exp(s)
            p = e / e.sum(axis=-1, keepdims=True)
            yb[:, hi] = p @ v[bi, :, hi]
        out[bi] = yb.reshape(t, emb) @ Wo.T + bo
    return out


def kernel(x, mask, Wq, bq, Wk, bk, Wv, bv, Wo, bo):
    global _CACHED_NC, LAST_RESULTS
    x = np.asarray(x, dtype=np.float32)
    mask = np.asarray(mask)
    Wq, Wk, Wv, Wo = (np.asarray(w, dtype=np.float32) for w in (Wq, Wk, Wv, Wo))
    bq, bk, bv, bo = (np.asarray(v_, dtype=np.float32) for v_ in (bq, bk, bv, bo))

    # The device program hardcodes a causal mask and zero q/k/v biases
    # (which is what reference.setup_inputs produces).  Anything else falls
    # back to a plain numpy implementation.
    tril = np.tril(np.ones((T, T), dtype=mask.dtype))
    if (
        x.shape != (B, T, EMB)
        or any(np.any(bias) for bias in (bq, bk, bv))
        or not all(np.array_equal(np.asarray(mask[b_]), tril) for b_ in range(B))
    ):
        return _numpy_fallback(x, mask, Wq, bq, Wk, bk, Wv, bv, Wo, bo)

    from concourse import bass_utils

    f16 = np.float16
    xT = [np.ascontiguousarray(x[b_].T).astype(f16) for b_ in range(B)]
    cmask = _causal_mask_tiles()
    in_maps = []
    for c in range(NCORES):
        b_, hg = c // 2, c % 2
        r = slice(hg * DPC, (hg + 1) * DPC)
        in_maps.append({
            "xT": xT[b_],
            "wqT": np.ascontiguousarray(Wq[r, :].T).astype(f16),
            "wkT": np.ascontiguousarray(Wk[r, :].T).astype(f16),
            "wvT": np.ascontiguousarray(Wv[r, :].T).astype(f16),
            "woT": np.ascontiguousarray(Wo[:, r].T).astype(f16),
            "cmask": cmask,
        })

    if _CACHED_NC is None:
        _CACHED_NC = _build_nc()

    import os
    trace = bool(int(os.environ.get("KERNEL_TRACE", "0")))
    res = bass_utils.run_bass_kernel_spmd(
        _CACHED_NC,
        in_maps,
        core_ids=list(range(NCORES)),
        trace=trace,
    )
    LAST_RESULTS = res
    outs = [r["out"] for r in res.results]
    y = np.stack([outs[2 * b_] + outs[2 * b_ + 1] for b_ in range(B)])
    y += bo[None, None, :]
    return np.ascontiguousarray(y.astype(np.float32))
